# revision 1
# baseline (speedup 1.0000x reference)
"""Pairwise IoU (8192x8192) on 8 Trainium2 NeuronCores via Bass/Tile.

Sharding: boxes1 rows split across 8 cores (1024 rows each); boxes2
replicated. Output row-sharded, gathered on host.

Per-core device kernel, per [128, MT] output tile:
  rx   = relu(min(x2_i, X2_j) - max(x1_i, X1_j))   1 fused custom-DVE op
  ry   = same for y                                 1 fused custom-DVE op
  usum = (area2_j + eps) + area1_i                  ACT (scalar engine)
  inter= rx*ry                                      DVE tensor_tensor
  u    = usum - inter                               DVE tensor_tensor
  rinv = reciprocal_approx_fast(u)  (~51 ULP)       1 custom-DVE op
  out  = inter * rinv                               DVE tensor_tensor
"""

import numpy as np

N = 8192
M = 8192
NCORES = 8
ROWS = N // NCORES  # rows of boxes1 per core
P = 128  # partitions
ITILES = ROWS // P  # 8 i-tiles per core
MT = 1024  # j-chunk width
JCHUNKS = M // MT
EPS = 1e-7

_COMPILED = {}


def _ensure_iou_edge():
    """Register the IOU_EDGE custom DVE op (idempotent)."""
    import concourse.dve_ops as dve_ops

    if any(op.name == "IOU_EDGE" for op in dve_ops.OPS):
        return next(op for op in dve_ops.OPS if op.name == "IOU_EDGE")

    from concourse.dve_spec import Spec, Src0, Src1, C0, C1, relu, minn, maxx

    spec = Spec(
        body=relu(minn(Src1, C1) - maxx(Src0, C0)),
        reference=lambda in0, in1, s0, s1, imm2: np.maximum(
            np.minimum(in1, s1) - np.maximum(in0, s0), 0.0
        ).astype(np.float32),
    )
    op = dve_ops.DveOp(
        "IOU_EDGE",
        spec,
        subdim=False,
        uops_sha={"v3": "6891eb10878e1367", "v4": "ef621f43a8326356"},
    )
    dve_ops.OPS.append(op)
    dve_ops.CUSTOM_DVE_SPECS[op.name] = op.spec
    dve_ops._SUB_OPCODE_FOR_NAME[op.name] = (
        dve_ops._CUSTOM_DVE_ROW_BASE + len(dve_ops.OPS) - 1
    )
    return op


def _build_program():
    from contextlib import ExitStack

    import concourse.bacc as bacc
    import concourse.mybir as mybir
    import concourse.tile as tile

    iou_edge = _ensure_iou_edge()

    f32 = mybir.dt.float32
    nc = bacc.Bacc(
        "TRN2",
        target_bir_lowering=False,
        debug=False,
        enable_asserts=False,
        num_devices=NCORES,
    )

    # DRAM I/O. Broadcast tensors are host-replicated across partitions.
    x1b = nc.dram_tensor("x1b", [P, M], f32, kind="ExternalInput").ap()
    x2b = nc.dram_tensor("x2b", [P, M], f32, kind="ExternalInput").ap()
    y1b = nc.dram_tensor("y1b", [P, M], f32, kind="ExternalInput").ap()
    y2b = nc.dram_tensor("y2b", [P, M], f32, kind="ExternalInput").ap()
    a2eb = nc.dram_tensor("a2eb", [P, M], f32, kind="ExternalInput").ap()
    # Per-partition scalars: for i-tile t, columns t*5+k hold
    # (x1, x2, y1, y2, area1) of boxes1 row t*128+p.
    sc = nc.dram_tensor("sc", [P, ITILES * 5], f32, kind="ExternalInput").ap()
    out = nc.dram_tensor("out", [ROWS, M], f32, kind="ExternalOutput").ap()

    with tile.TileContext(nc) as tc, ExitStack() as ctx:
        bcast = ctx.enter_context(tc.tile_pool(name="bcast", bufs=2))
        scp = ctx.enter_context(tc.tile_pool(name="scp", bufs=1))
        work = ctx.enter_context(tc.tile_pool(name="work", bufs=2))
        outp = ctx.enter_context(tc.tile_pool(name="outp", bufs=3))

        sct = scp.tile([P, ITILES * 5], f32)
        nc.sync.dma_start(sct[:], sc[:])

        for j in range(JCHUNKS):
            j0 = j * MT
            x1c = bcast.tile([P, MT], f32, tag="x1c")
            x2c = bcast.tile([P, MT], f32, tag="x2c")
            y1c = bcast.tile([P, MT], f32, tag="y1c")
            y2c = bcast.tile([P, MT], f32, tag="y2c")
            a2ec = bcast.tile([P, MT], f32, tag="a2ec")
            nc.sync.dma_start(x1c[:], x1b[:, j0 : j0 + MT])
            nc.sync.dma_start(x2c[:], x2b[:, j0 : j0 + MT])
            nc.sync.dma_start(y1c[:], y1b[:, j0 : j0 + MT])
            nc.sync.dma_start(y2c[:], y2b[:, j0 : j0 + MT])
            nc.sync.dma_start(a2ec[:], a2eb[:, j0 : j0 + MT])

            for t in range(ITILES):
                c = t * 5
                s_x1 = sct[:, c : c + 1]
                s_x2 = sct[:, c + 1 : c + 2]
                s_y1 = sct[:, c + 2 : c + 3]
                s_y2 = sct[:, c + 3 : c + 4]
                s_a1 = sct[:, c + 4 : c + 5]

                rx = work.tile([P, MT], f32, tag="rx")
                ry = work.tile([P, MT], f32, tag="ry")
                usum = work.tile([P, MT], f32, tag="usum")
                inter = work.tile([P, MT], f32, tag="inter")
                u = work.tile([P, MT], f32, tag="u")
                rinv = work.tile([P, MT], f32, tag="rinv")
                ot = outp.tile([P, MT], f32, tag="ot")

                nc.vector._custom_dve(
                    iou_edge, out=rx[:], in0=x1c[:], in1=x2c[:], s0=s_x1, s1=s_x2
                )
                nc.vector._custom_dve(
                    iou_edge, out=ry[:], in0=y1c[:], in1=y2c[:], s0=s_y1, s1=s_y2
                )
                nc.scalar.activation(
                    usum[:],
                    a2ec[:],
                    mybir.ActivationFunctionType.Identity,
                    bias=s_a1,
                    scale=1.0,
                )
                nc.vector.tensor_mul(inter[:], rx[:], ry[:])
                nc.vector.tensor_sub(u[:], usum[:], inter[:])
                nc.vector.reciprocal_approx_fast(rinv[:], u[:])
                nc.vector.tensor_mul(ot[:], inter[:], rinv[:])
                nc.sync.dma_start(out[t * P : (t + 1) * P, j0 : j0 + MT], ot[:])

    nc.compile()
    return nc


def _get_program():
    if "nc" not in _COMPILED:
        _COMPILED["nc"] = _build_program()
    return _COMPILED["nc"]


def kernel(boxes1: np.ndarray, boxes2: np.ndarray) -> np.ndarray:
    from concourse.bass_utils import run_bass_kernel_spmd

    nc = _get_program()

    boxes1 = np.ascontiguousarray(boxes1, dtype=np.float32)
    boxes2 = np.ascontiguousarray(boxes2, dtype=np.float32)

    # Host-side prep (O(N)): replicated boxes2 component rows + area2.
    a2e = (boxes2[:, 2] - boxes2[:, 0]) * (boxes2[:, 3] - boxes2[:, 1]) + np.float32(
        EPS
    )
    reps = {}
    for name, vec in (
        ("x1b", boxes2[:, 0]),
        ("x2b", boxes2[:, 2]),
        ("y1b", boxes2[:, 1]),
        ("y2b", boxes2[:, 3]),
        ("a2eb", a2e),
    ):
        reps[name] = np.ascontiguousarray(
            np.broadcast_to(vec.astype(np.float32), (P, M))
        )

    in_maps = []
    for c in range(NCORES):
        b1c = boxes1[c * ROWS : (c + 1) * ROWS].reshape(ITILES, P, 4)
        a1 = (b1c[:, :, 2] - b1c[:, :, 0]) * (b1c[:, :, 3] - b1c[:, :, 1])
        sc = np.empty((P, ITILES * 5), dtype=np.float32)
        for t in range(ITILES):
            sc[:, t * 5 + 0] = b1c[t, :, 0]
            sc[:, t * 5 + 1] = b1c[t, :, 2]
            sc[:, t * 5 + 2] = b1c[t, :, 1]
            sc[:, t * 5 + 3] = b1c[t, :, 3]
            sc[:, t * 5 + 4] = a1[t]
        in_maps.append({**reps, "sc": sc})

    res = run_bass_kernel_spmd(nc, in_maps, core_ids=list(range(NCORES)))
    return np.concatenate([res.results[c]["out"] for c in range(NCORES)], axis=0)


# revision 3
# speedup vs baseline: 1.0723x; 1.0723x over previous
"""Pairwise IoU (8192x8192) on 8 Trainium2 NeuronCores via Bass/Tile.

Sharding: boxes1 rows split across 8 cores (1024 rows each); boxes2
replicated. Output row-sharded, gathered on host.

Per-core device kernel, per [128, MT] output tile (5 DVE passes):
  rx    = relu(min(x2_i, X2_j) - max(x1_i, X1_j))   fused custom-DVE op
  ry    = same for y                                fused custom-DVE op
  inter = rx*ry                                     DVE tensor_tensor
  u     = a1_i + (a2_j+eps) - inter                 TENSOR ENGINE -> PSUM
          (rank-1 matmul + (-I) @ inter accumulate; no DVE cost)
  rinv  = reciprocal_approx_fast(u_psum)            custom-DVE op (~51 ULP)
  out   = inter * rinv                              DVE tensor_tensor
"""

import numpy as np

N = 8192
M = 8192
NCORES = 8
ROWS = N // NCORES  # rows of boxes1 per core
P = 128  # partitions
ITILES = ROWS // P  # 8 i-tiles per core
MT = 2048  # j-chunk width
JCHUNKS = M // MT
PS = 512  # psum bank width (fp32)
PCHUNKS = MT // PS
EPS = 1e-7

_COMPILED = {}


def _ensure_iou_edge():
    """Register the IOU_EDGE custom DVE op (idempotent)."""
    import concourse.dve_ops as dve_ops

    for op in dve_ops.OPS:
        if op.name == "IOU_EDGE":
            return op

    from concourse.dve_spec import Spec, Src0, Src1, C0, C1, relu, minn, maxx

    spec = Spec(
        body=relu(minn(Src1, C1) - maxx(Src0, C0)),
        reference=lambda in0, in1, s0, s1, imm2: np.maximum(
            np.minimum(in1, s1) - np.maximum(in0, s0), 0.0
        ).astype(np.float32),
    )
    op = dve_ops.DveOp(
        "IOU_EDGE",
        spec,
        subdim=False,
        uops_sha={"v3": "6891eb10878e1367", "v4": "ef621f43a8326356"},
    )
    dve_ops.OPS.append(op)
    dve_ops.CUSTOM_DVE_SPECS[op.name] = op.spec
    dve_ops._SUB_OPCODE_FOR_NAME[op.name] = (
        dve_ops._CUSTOM_DVE_ROW_BASE + len(dve_ops.OPS) - 1
    )
    return op


def _build_program():
    from contextlib import ExitStack

    import concourse.bacc as bacc
    import concourse.mybir as mybir
    import concourse.tile as tile
    from concourse.dve_ops import RECIPROCAL_APPROX_FAST, RECIP_APPROX_FAST_CONSTS

    iou_edge = _ensure_iou_edge()
    rc = RECIP_APPROX_FAST_CONSTS

    f32 = mybir.dt.float32
    nc = bacc.Bacc(
        "TRN2",
        target_bir_lowering=False,
        debug=False,
        enable_asserts=False,
        num_devices=NCORES,
    )

    # DRAM I/O. Broadcast tensors are host-replicated across partitions.
    x1b = nc.dram_tensor("x1b", [P, M], f32, kind="ExternalInput").ap()
    x2b = nc.dram_tensor("x2b", [P, M], f32, kind="ExternalInput").ap()
    y1b = nc.dram_tensor("y1b", [P, M], f32, kind="ExternalInput").ap()
    y2b = nc.dram_tensor("y2b", [P, M], f32, kind="ExternalInput").ap()
    # moving operand for the union matmul: row0 = ones, row1 = area2+eps
    a2e2 = nc.dram_tensor("a2e2", [2, M], f32, kind="ExternalInput").ap()
    # stationary for the union matmul: row0 = area1 (per row), row1 = ones
    sta = nc.dram_tensor("sta", [2, ROWS], f32, kind="ExternalInput").ap()
    # negated identity for the -inter accumulate
    negi = nc.dram_tensor("negi", [P, P], f32, kind="ExternalInput").ap()
    # Per-partition scalars: for i-tile t, columns t*4+k hold
    # (x1, x2, y1, y2) of boxes1 row t*128+p.
    sc = nc.dram_tensor("sc", [P, ITILES * 4], f32, kind="ExternalInput").ap()
    out = nc.dram_tensor("out", [ROWS, M], f32, kind="ExternalOutput").ap()

    with tile.TileContext(nc) as tc, ExitStack() as ctx:
        bcast = ctx.enter_context(tc.tile_pool(name="bcast", bufs=2))
        scp = ctx.enter_context(tc.tile_pool(name="scp", bufs=1))
        work = ctx.enter_context(tc.tile_pool(name="work", bufs=2))
        outp = ctx.enter_context(tc.tile_pool(name="outp", bufs=3))
        psum = ctx.enter_context(
            tc.tile_pool(name="psum", bufs=8, space="PSUM")
        )

        sct = scp.tile([P, ITILES * 4], f32)
        nc.sync.dma_start(sct[:], sc[:])
        stat = scp.tile([2, ROWS], f32)
        nc.sync.dma_start(stat[:], sta[:])
        negit = scp.tile([P, P], f32)
        nc.sync.dma_start(negit[:], negi[:])
        a2e2t = scp.tile([2, M], f32)
        nc.sync.dma_start(a2e2t[:], a2e2[:])

        for j in range(JCHUNKS):
            j0 = j * MT
            x1c = bcast.tile([P, MT], f32, tag="x1c")
            x2c = bcast.tile([P, MT], f32, tag="x2c")
            y1c = bcast.tile([P, MT], f32, tag="y1c")
            y2c = bcast.tile([P, MT], f32, tag="y2c")
            nc.sync.dma_start(x1c[:], x1b[:, j0 : j0 + MT])
            nc.sync.dma_start(x2c[:], x2b[:, j0 : j0 + MT])
            nc.sync.dma_start(y1c[:], y1b[:, j0 : j0 + MT])
            nc.sync.dma_start(y2c[:], y2b[:, j0 : j0 + MT])

            for t in range(ITILES):
                c = t * 4
                s_x1 = sct[:, c : c + 1]
                s_x2 = sct[:, c + 1 : c + 2]
                s_y1 = sct[:, c + 2 : c + 3]
                s_y2 = sct[:, c + 3 : c + 4]

                rx = work.tile([P, MT], f32, tag="rx")
                ry = work.tile([P, MT], f32, tag="ry")
                inter = work.tile([P, MT], f32, tag="inter")
                rinv = work.tile([P, MT], f32, tag="rinv")
                ot = outp.tile([P, MT], f32, tag="ot")

                nc.vector._custom_dve(
                    iou_edge, out=rx[:], in0=x1c[:], in1=x2c[:], s0=s_x1, s1=s_x2
                )
                nc.vector._custom_dve(
                    iou_edge, out=ry[:], in0=y1c[:], in1=y2c[:], s0=s_y1, s1=s_y2
                )
                nc.vector.tensor_mul(inter[:], rx[:], ry[:])

                for pc in range(PCHUNKS):
                    c0 = pc * PS
                    ps = psum.tile([P, PS], f32, tag="ps")
                    # u = a1_i + (a2_j + eps) ...
                    nc.tensor.matmul(
                        ps[:],
                        stat[:, t * P : (t + 1) * P],
                        a2e2t[:, j0 + c0 : j0 + c0 + PS],
                        start=True,
                        stop=False,
                    )
                    # ... - inter
                    nc.tensor.matmul(
                        ps[:],
                        negit[:],
                        inter[:, c0 : c0 + PS],
                        start=False,
                        stop=True,
                    )
                    nc.vector._custom_dve(
                        RECIPROCAL_APPROX_FAST,
                        out=rinv[:, c0 : c0 + PS],
                        in0=ps[:],
                        s0=rc["s0"],
                        s1=rc["s1"],
                        imm2=rc["imm2"],
                    )

                nc.vector.tensor_mul(ot[:], inter[:], rinv[:])
                nc.sync.dma_start(out[t * P : (t + 1) * P, j0 : j0 + MT], ot[:])

    nc.compile()
    return nc


def _get_program():
    if "nc" not in _COMPILED:
        _COMPILED["nc"] = _build_program()
    return _COMPILED["nc"]


def _make_in_maps(boxes1, boxes2):
    boxes1 = np.ascontiguousarray(boxes1, dtype=np.float32)
    boxes2 = np.ascontiguousarray(boxes2, dtype=np.float32)

    a2e = (boxes2[:, 2] - boxes2[:, 0]) * (boxes2[:, 3] - boxes2[:, 1]) + np.float32(
        EPS
    )
    reps = {}
    for name, vec in (
        ("x1b", boxes2[:, 0]),
        ("x2b", boxes2[:, 2]),
        ("y1b", boxes2[:, 1]),
        ("y2b", boxes2[:, 3]),
    ):
        reps[name] = np.ascontiguousarray(
            np.broadcast_to(vec.astype(np.float32), (P, M))
        )
    a2e2 = np.stack([np.ones(M, np.float32), a2e]).astype(np.float32)
    negi = (-np.eye(P)).astype(np.float32)

    in_maps = []
    for c in range(NCORES):
        b1c = boxes1[c * ROWS : (c + 1) * ROWS].reshape(ITILES, P, 4)
        a1 = (b1c[:, :, 2] - b1c[:, :, 0]) * (b1c[:, :, 3] - b1c[:, :, 1])
        sta = np.stack(
            [a1.reshape(ROWS), np.ones(ROWS, np.float32)]
        ).astype(np.float32)
        sc = np.empty((P, ITILES * 4), dtype=np.float32)
        for t in range(ITILES):
            sc[:, t * 4 + 0] = b1c[t, :, 0]
            sc[:, t * 4 + 1] = b1c[t, :, 2]
            sc[:, t * 4 + 2] = b1c[t, :, 1]
            sc[:, t * 4 + 3] = b1c[t, :, 3]
        in_maps.append(
            {**reps, "a2e2": a2e2, "sta": sta, "negi": negi, "sc": sc}
        )
    return in_maps


def kernel(boxes1: np.ndarray, boxes2: np.ndarray) -> np.ndarray:
    from concourse.bass_utils import run_bass_kernel_spmd

    nc = _get_program()
    in_maps = _make_in_maps(boxes1, boxes2)
    res = run_bass_kernel_spmd(nc, in_maps, core_ids=list(range(NCORES)))
    return np.concatenate([res.results[c]["out"] for c in range(NCORES)], axis=0)
